# revision 16
# baseline (speedup 1.0000x reference)
# CCAM channel-attention kernel for Trainium2 (Bass/Tile), 8-core SPMD.
#
# Math (per batch b):
#   q = x[b].reshape(C, N)                      # N = H*W = 4096
#   energy = q @ kbank                          # (C, 64), kbank = martx[0]
#   att = softmax(aphal * (rowmax(energy) - energy), axis=-1)
#   out = gamma * (att @ kbank.T) + x[b]
#
# Sharding: data-parallel over batch B=16 across 8 cores (2 batches/core);
# kbank, aphal, gamma are replicated.  aphal/gamma are baked into the
# program as immediates (cache keyed on their values).
#
# Per-core layout: the 2048 (b,c) rows are processed in 16 tiles of 128
# rows.  The contraction of matmul-1 runs over n, so q must be transposed
# on-chip: 32 PE transposes (fp32) per tile, cast to bf16 during the
# mandatory PSUM->SBUF copy (ScalarE).  Both matmuls run in bf16 (the
# attention output is a small residual correction to x, so bf16 error is
# negligible in the final fp32 output).  Softmax normalization and gamma
# are folded into the fused (psum * (gamma/s)) + x residual op on DVE.

import numpy as np
from contextlib import ExitStack

B, C = 16, 1024
HW = 4096          # H*W
KD = 64            # key bank dim
N_CORES = 8
P = 128            # partitions
ROWS = (B // N_CORES) * C   # 2048 rows per core
NT = ROWS // P              # 16 row tiles per core
NCH = HW // P               # 32 contraction chunks
NF = HW // 512              # 8 output free-dim chunks

_programs = {}


def _build_program(aphal: float, gamma: float, cfg: dict | None = None):
    cfg = cfg or {}
    xs_bufs = cfg.get("xs_bufs", 5)
    qts_bufs = cfg.get("qts_bufs", 2)
    outs_bufs = cfg.get("outs_bufs", 2)
    pst_bufs = cfg.get("pst_bufs", 3)
    pse_bufs = cfg.get("pse_bufs", 1)
    psa_bufs = cfg.get("psa_bufs", 2)
    pso_bufs = cfg.get("pso_bufs", 2)
    split_in = cfg.get("split_in", 1)    # x load split per tile
    split_out = cfg.get("split_out", 1)  # out store split per tile
    dma_only = cfg.get("dma_only", False)  # timing-study mutant: no compute
    prefetch = cfg.get("prefetch", 3)    # x loads emitted this many tiles ahead
    qt_chunk = cfg.get("qt_chunk", 4)    # transposes per PSUM group (4 or 8)
    res_chunk = cfg.get("res_chunk", 4)  # 128-col blocks per residual op (4 or 8)
    import concourse.mybir as mybir
    import concourse.tile as tile
    from concourse import bacc
    from concourse.masks import make_identity

    f32 = mybir.dt.float32
    bf16 = mybir.dt.bfloat16

    nc = bacc.Bacc(
        "TRN2",
        target_bir_lowering=False,
        debug=False,
        enable_asserts=False,
        num_devices=N_CORES,
    )
    x_d = nc.dram_tensor("x", (ROWS, HW), f32, kind="ExternalInput").ap()
    kb_d = nc.dram_tensor("kb", (HW, KD), f32, kind="ExternalInput").ap()
    out_d = nc.dram_tensor("out", (ROWS, HW), f32, kind="ExternalOutput").ap()

    with tile.TileContext(nc) as tc, ExitStack() as ctx:
        const = ctx.enter_context(tc.tile_pool(name="const", bufs=1))
        xs = ctx.enter_context(tc.tile_pool(name="xs", bufs=xs_bufs))
        qts = ctx.enter_context(tc.tile_pool(name="qts", bufs=qts_bufs))
        outs = ctx.enter_context(tc.tile_pool(name="outs", bufs=outs_bufs))
        small = ctx.enter_context(tc.tile_pool(name="small", bufs=6))
        ps_t = ctx.enter_context(tc.tile_pool(name="ps_t", bufs=pst_bufs, space="PSUM"))
        ps_e = ctx.enter_context(tc.tile_pool(name="ps_e", bufs=pse_bufs, space="PSUM"))
        ps_a = ctx.enter_context(tc.tile_pool(name="ps_a", bufs=psa_bufs, space="PSUM"))
        ps_o = ctx.enter_context(tc.tile_pool(name="ps_o", bufs=pso_bufs, space="PSUM"))

        ident32 = const.tile([P, P], f32)
        make_identity(nc, ident32)
        ident16 = const.tile([P, P], bf16)
        make_identity(nc, ident16)

        # kbank in chunked layout: kb_sb[p, a, k] = kbank[a*128 + p, k]
        kb_sb = const.tile([P, NCH, KD], f32)
        nc.sync.dma_start(out=kb_sb, in_=kb_d.rearrange("(a p) k -> p a k", p=P))
        kb16 = const.tile([P, NCH, KD], bf16)
        nc.vector.tensor_copy(kb16, kb_sb)

        # kbank^T in bf16: kbT16[k, n]
        kbT16 = const.tile([KD, HW], bf16)
        for a in range(NCH):
            pst = ps_a.tile([KD, P], f32, tag="psa")
            nc.tensor.transpose(pst, kb_sb[:, a, :], ident32)
            nc.scalar.copy(kbT16[:, a * P:(a + 1) * P], pst)

        xts = {}

        def load_x(t):
            xt = xs.tile([P, NCH, P], f32)
            x_src = x_d[t * P:(t + 1) * P, :].rearrange("p (a q) -> p a q", q=P)
            ci = NCH // split_in
            for s in range(split_in):
                nc.sync.dma_start(
                    out=xt[:, s * ci:(s + 1) * ci, :],
                    in_=x_src[:, s * ci:(s + 1) * ci, :],
                )
            xts[t] = xt

        for t in range(min(prefetch, NT)):
            load_x(t)

        for t in range(NT):
            # --- load x tile (128 rows x 4096) ---
            if t + prefetch < NT:
                load_x(t + prefetch)
            elif t not in xts:
                load_x(t)
            xt = xts.pop(t)

            if dma_only:
                o_dst = out_d[t * P:(t + 1) * P, :].rearrange(
                    "p (a q) -> p a q", q=P
                )
                nc.sync.dma_start(out=o_dst, in_=xt)
                continue

            # --- transpose q: 32 PE transposes, qt_chunk per PSUM group, cast bf16 ---
            qT16 = qts.tile([P, NCH, P], bf16)
            for g in range(NCH // qt_chunk):
                psq = ps_t.tile([P, qt_chunk, P], f32)
                for j in range(qt_chunk):
                    a = qt_chunk * g + j
                    nc.tensor.transpose(psq[:, j, :], xt[:, a, :], ident32)
                nc.scalar.copy(
                    qT16[:, qt_chunk * g:qt_chunk * (g + 1), :], psq
                )

            # --- energy = q @ kbank : accumulate over 32 chunks ---
            pse = ps_e.tile([P, KD], f32)
            for a in range(NCH):
                nc.tensor.matmul(
                    pse,
                    lhsT=qT16[:, a, :],
                    rhs=kb16[:, a, :],
                    start=(a == 0),
                    stop=(a == NCH - 1),
                )

            # --- inverted softmax: exp(aphal*(max - e)), unnormalized ---
            mx = small.tile([P, 1], f32)
            nc.vector.reduce_max(mx, pse, axis=mybir.AxisListType.X)
            mxs = small.tile([P, 1], f32)
            nc.vector.tensor_scalar_mul(mxs, mx, float(aphal))
            att16 = small.tile([P, KD], bf16)
            ssum = small.tile([P, 1], f32)
            nc.scalar.activation(
                att16,
                pse,
                mybir.ActivationFunctionType.Exp,
                bias=mxs,
                scale=-float(aphal),
                accum_out=ssum,
            )
            rinv = small.tile([P, 1], f32)
            nc.vector.reciprocal(rinv, ssum)
            rg = small.tile([P, 1], f32)
            nc.vector.tensor_scalar_mul(rg, rinv, float(gamma))

            # --- att^T (PE transpose, bf16) ---
            psa = ps_a.tile([KD, P], bf16, tag="psa")
            nc.tensor.transpose(psa, att16, ident16)
            attT = small.tile([KD, P], bf16)
            nc.scalar.copy(attT, psa)

            # --- out = (att @ kbank^T) * (gamma/s) + x ;  DMA out ---
            ot = outs.tile([P, NCH, P], f32)
            mm_per_res = res_chunk // 4  # matmuls (N=512) per residual op
            for r in range(NCH // res_chunk):
                pso = ps_o.tile([P, res_chunk, P], f32)
                for m in range(mm_per_res):
                    nf = r * mm_per_res + m
                    nc.tensor.matmul(
                        pso[:, 4 * m:4 * (m + 1), :],
                        lhsT=attT,
                        rhs=kbT16[:, nf * 512:(nf + 1) * 512],
                        start=True,
                        stop=True,
                    )
                nc.vector.scalar_tensor_tensor(
                    out=ot[:, res_chunk * r:res_chunk * (r + 1), :],
                    in0=pso,
                    scalar=rg,
                    in1=xt[:, res_chunk * r:res_chunk * (r + 1), :],
                    op0=mybir.AluOpType.mult,
                    op1=mybir.AluOpType.add,
                )
            o_dst = out_d[t * P:(t + 1) * P, :].rearrange("p (a q) -> p a q", q=P)
            co = NCH // split_out
            for s in range(split_out):
                nc.sync.dma_start(
                    out=o_dst[:, s * co:(s + 1) * co, :],
                    in_=ot[:, s * co:(s + 1) * co, :],
                )

    nc.compile()
    return nc


def _get_program(aphal: float, gamma: float):
    key = (aphal, gamma)
    if key not in _programs:
        _programs[key] = _build_program(aphal, gamma)
    return _programs[key]


def run(x, martx, aphal, gamma, trace=False):
    """Returns (output, BassKernelResults)."""
    from concourse.bass_utils import run_bass_kernel_spmd
    from concourse.bass_interp import get_hw_module

    x = np.ascontiguousarray(np.asarray(x, dtype=np.float32))
    kb = np.ascontiguousarray(
        np.asarray(martx, dtype=np.float32).reshape(HW, KD)
    )
    a_val = float(np.asarray(aphal).reshape(-1)[0])
    g_val = float(np.asarray(gamma).reshape(-1)[0])

    nc = _get_program(a_val, g_val)
    shards = x.reshape(N_CORES, ROWS, HW)
    in_maps = [{"x": shards[i], "kb": kb} for i in range(N_CORES)]

    old_m = nc.m
    nc.m = get_hw_module(nc.m)
    try:
        res = run_bass_kernel_spmd(
            nc, in_maps, core_ids=list(range(N_CORES)), trace=trace
        )
    finally:
        nc.m = old_m

    out = np.stack([res.results[i]["out"] for i in range(N_CORES)])
    out = out.reshape(B, C, 64, 64).astype(np.float32)
    return out, res


def kernel(x, martx, aphal, gamma):
    out, _ = run(x, martx, aphal, gamma, trace=False)
    return out
